# revision 3
# baseline (speedup 1.0000x reference)
"""Trainium2 Bass kernel for nn_Attention_10282151706748.

Computes, for B=32 batches over L=8192 context positions:
    q   = inputs @ Wi.T + bi                      (B, 128)
    k   = context @ Wh.T + bh                     (B, L, 128)
    att_row = tanh(q + k) @ V                     (B, 1, L)
    att = softmax over singleton dim -> ones      (B, L)

Strategy: data-parallel over batch across 8 NeuronCores (4 batches/core).
The tiny q projection (32x512x128) is computed on host and folded with the
biases; the device streams the 512MB context (host-transposed to put the
HID contraction dim on SBUF partitions) through the TensorEngine:
    per (b, l-tile of 512):  psum[d,l] = sum_j whT_j.T @ ctxT_j   (K=4x128)
    a = tanh(psum + qb[b])   on ACT (per-partition bias)
    att_row[l]  = V.T @ a    on PE  (M=1 matmul)
"""

import os
import sys

import numpy as np

if "/opt/trn_rl_repo" not in sys.path:
    sys.path.insert(0, "/opt/trn_rl_repo")

B, L, IN, HID, D3 = 32, 8192, 512, 512, 128
NCORES = 8
BLOC = B // NCORES          # 4 batches per core
LT = 512                    # l-tile (free dim of the k matmul)
NJ = HID // 128             # 4 contraction chunks
NLT = L // LT               # 16 l-tiles per batch

LAST_RESULT = None
_NC_CACHE = {}


def _build_nc():
    import concourse.bacc as bacc
    import concourse.mybir as mybir
    import concourse.tile as tile

    f32 = mybir.dt.float32
    nc = bacc.Bacc()

    ctx_t = nc.dram_tensor("ctx_t", [BLOC * HID, L], f32, kind="ExternalInput")
    qbT = nc.dram_tensor("qbT", [D3, BLOC], f32, kind="ExternalInput")
    whT = nc.dram_tensor("whT", [HID, D3], f32, kind="ExternalInput")
    vcol = nc.dram_tensor("vcol", [D3, 1], f32, kind="ExternalInput")
    out = nc.dram_tensor("out", [BLOC, L], f32, kind="ExternalOutput")

    # [BLOC, 128(par), NJ, L] view of the transposed context
    ctx_v = ctx_t[:, :].rearrange("(b j p) l -> b p j l", j=NJ, p=128)

    with tile.TileContext(nc) as tc:
        with (
            tc.tile_pool(name="consts", bufs=1) as consts,
            tc.tile_pool(name="cts", bufs=6) as cts,
            tc.tile_pool(name="asb", bufs=3) as asb,
            tc.tile_pool(name="outs", bufs=4) as outs,
            tc.tile_pool(name="pk", bufs=2, space="PSUM") as pk,
            tc.tile_pool(name="pr", bufs=2, space="PSUM") as pr,
        ):
            wh_sb = consts.tile([128, NJ, D3], f32)
            nc.sync.dma_start(
                out=wh_sb, in_=whT[:, :].rearrange("(j p) d -> p j d", p=128)
            )
            qb_sb = consts.tile([128, BLOC], f32)
            nc.sync.dma_start(out=qb_sb, in_=qbT[:, :])
            v_sb = consts.tile([128, 1], f32)
            nc.sync.dma_start(out=v_sb, in_=vcol[:, :])

            for b in range(BLOC):
                for lt in range(NLT):
                    ct = cts.tile([128, NJ, LT], f32, tag="ct")
                    nc.sync.dma_start(
                        out=ct, in_=ctx_v[b, :, :, lt * LT:(lt + 1) * LT]
                    )
                    ps = pk.tile([128, LT], f32, tag="ps")
                    for j in range(NJ):
                        nc.tensor.matmul(
                            ps,
                            wh_sb[:, j, :],
                            ct[:, j, :],
                            start=(j == 0),
                            stop=(j == NJ - 1),
                        )
                    a = asb.tile([128, LT], f32, tag="a")
                    nc.scalar.activation(
                        out=a,
                        in_=ps,
                        func=mybir.ActivationFunctionType.Tanh,
                        bias=qb_sb[:, b:b + 1],
                        scale=1.0,
                    )
                    pv = pr.tile([1, LT], f32, tag="pv")
                    nc.tensor.matmul(pv, v_sb, a, start=True, stop=True)
                    o = outs.tile([1, LT], f32, tag="o")
                    nc.vector.tensor_copy(o, pv)
                    nc.scalar.dma_start(
                        out=out[b:b + 1, lt * LT:(lt + 1) * LT], in_=o
                    )
    nc.compile()
    return nc


def _get_nc():
    if "nc" not in _NC_CACHE:
        _NC_CACHE["nc"] = _build_nc()
    return _NC_CACHE["nc"]


def kernel(inputs, context, Wi, bi, Wh, bh, V):
    global LAST_RESULT
    from concourse.bass_utils import run_bass_kernel_spmd

    inputs = np.asarray(inputs, np.float32)
    context = np.asarray(context, np.float32)
    Wi = np.asarray(Wi, np.float32)
    bi = np.asarray(bi, np.float32)
    Wh = np.asarray(Wh, np.float32)
    bh = np.asarray(bh, np.float32)
    V = np.asarray(V, np.float32)

    # Host-side prep: fold the small q projection and both biases into a
    # per-(batch, d) additive term; transpose context so HID lands on the
    # SBUF partition (contraction) dim with L contiguous in DRAM.
    qb = inputs @ Wi.T + bi + bh                     # (B, D3)
    qbT = np.ascontiguousarray(qb.T, np.float32)     # (D3, B)
    whT = np.ascontiguousarray(Wh.T, np.float32)     # (HID, D3)
    vcol = np.ascontiguousarray(V.reshape(D3, 1), np.float32)
    ctx_t = np.ascontiguousarray(context.transpose(0, 2, 1))  # (B, HID, L)

    nc = _get_nc()
    in_maps = []
    for c in range(NCORES):
        in_maps.append(
            {
                "ctx_t": ctx_t[c * BLOC:(c + 1) * BLOC].reshape(BLOC * HID, L),
                "qbT": np.ascontiguousarray(qbT[:, c * BLOC:(c + 1) * BLOC]),
                "whT": whT,
                "vcol": vcol,
            }
        )

    res = run_bass_kernel_spmd(
        nc,
        in_maps,
        list(range(NCORES)),
        trace=bool(os.environ.get("KTRACE")),
    )
    LAST_RESULT = res

    att_row = np.concatenate(
        [res.results[c]["out"] for c in range(NCORES)], axis=0
    )  # (B, L)
    att = np.ones((B, L), np.float32)
    return att_row[:, None, :], att
